# revision 1
# baseline (speedup 1.0000x reference)
"""MoE layer (dense all-experts SwiGLU + router-weighted sum) on 8 TRN2 cores.

Expert-parallel: core e holds expert e's weights (E=8). Every core sees the
full token stream x (shipped pre-transposed as xT [H, N]) and computes
  y_e = softmax(x @ W_router)[:, e] * ((silu(x@Wg_e) * (x@Wu_e)) @ Wd_e)
The host sums the 8 per-expert outputs.

Matmuls run in float32r (TF32-like: full-rate PE, ~1.5e-4 rel err), PSUM
accumulation in fp32, all vector/scalar ops in fp32.

Per-core program, per 512-token block:
  router: logits^T [8,512] via PE (Wr stationary, xT moving), Exp on ACT,
          then per 128-token subtile a transpose-matmul with rhs=[ones|e_sel]
          gives [denom | numer] in PSUM -> w = numer * 1/denom on DVE.
  stage1: G/U [128i, 512tok] = Wg/Wu_chunk^T @ xT_chunk (8 K-chunks in PSUM),
          hT[i] = silu(G)*U -> SBUF (resident for the block, [I, tok] layout).
  stage2: Y[m] [128tok, 512h] accumulates over 16 i-chunks with hT as
          stationary and streamed Wd 512KB i-pair tiles as moving; evict =
          DVE multiply by the router weight, y DMAs deferred to block end.

Scheduling notes (hard-won):
  - xt rides the gpsimd (SWDGE) queue so block b+1's chunks never queue
    behind stage-2's wd stream on sync (HWDGE); bulk wd on SWDGE is slow.
  - next block's router runs between the two h-sweeps (PE filler + takes
    it off the block-start critical path).
  - block 0 splits stage 1 into a G-pass then U-pass so compute starts
    as soon as Wg lands (Wg is streamed before Wu).
  - tile-pool slot allocation order must match consumption order or the
    schedule deadlocks; all 8 xt chunks of a block are live at once.
"""
import numpy as np

import concourse.bass as bass
import concourse.mybir as mybir
import concourse.tile as tile
from concourse import bacc
from concourse.bass_utils import run_bass_kernel_spmd

P = 128
H, I, E = 1024, 2048, 8
N = 8192  # tokens = 4 * 2048
HK = H // P   # 8 contraction chunks over H
IK = I // P   # 16 chunks over I
TB = 512      # token block
NB = N // TB  # 16 blocks
NM = TB // P  # 4 token subtiles per block
NH = H // 512  # 2 output column halves

F32 = mybir.dt.float32
F32R = mybir.dt.float32r
AF = mybir.ActivationFunctionType

# set by a driver (test.py) to profile; harness path keeps defaults
TRACE = False
LAST_EXEC_NS = None

_CACHE = {}


def _build():
    nc = bacc.Bacc("TRN2", target_bir_lowering=False, debug=False)

    xt_d = nc.dram_tensor("xt", [H, N], F32R, kind="ExternalInput").ap()
    wg_d = nc.dram_tensor("wg", [H, I], F32R, kind="ExternalInput").ap()
    wu_d = nc.dram_tensor("wu", [H, I], F32R, kind="ExternalInput").ap()
    wd_d = nc.dram_tensor("wd", [I, H], F32R, kind="ExternalInput").ap()
    wr_d = nc.dram_tensor("wr", [H, E], F32R, kind="ExternalInput").ap()
    sel_d = nc.dram_tensor("sel", [E, 2], F32R, kind="ExternalInput").ap()
    y_d = nc.dram_tensor("y", [N, H], F32, kind="ExternalOutput").ap()

    with tile.TileContext(nc) as tc:
        with (
            tc.tile_pool(name="const", bufs=1) as const,
            tc.tile_pool(name="xtp", bufs=8) as xtp,
            tc.tile_pool(name="htp", bufs=1) as htp,
            tc.tile_pool(name="wdp", bufs=5) as wdp,
            tc.tile_pool(name="evp", bufs=4) as evp,
            tc.tile_pool(name="rtp", bufs=1) as rtp,
            tc.tile_pool(name="wp", bufs=2) as wp,
            tc.tile_pool(name="psgu", bufs=2, space="PSUM") as psgu,
            tc.tile_pool(name="psy", bufs=5, space="PSUM") as psy,
            tc.tile_pool(name="psr", bufs=1, space="PSUM") as psr,
        ):
            # resident weights: [128, HK*I] with chunk k at cols [k*I, (k+1)*I)
            wg_sb = const.tile([P, HK * I], F32R)
            wu_sb = const.tile([P, HK * I], F32R)
            wr_sb = const.tile([P, HK * E], F32R)
            sel_sb = const.tile([E, 2], F32R)
            # router inputs first (tiny) so router(0) starts early, then
            # wg/wu as 2MB 2-chunk batches (fewer DMA completion tails)
            nc.sync.dma_start(
                out=wr_sb[:].rearrange("p (k e) -> p k e", k=HK),
                in_=wr_d[:].rearrange("(k p) e -> p k e", p=P),
            )
            nc.sync.dma_start(out=sel_sb[:], in_=sel_d[:])
            for w_sb, w_d in ((wg_sb, wg_d), (wu_sb, wu_d)):
                for j in range(HK // 2):
                    rows = slice(2 * j * P, (2 * j + 2) * P)
                    nc.sync.dma_start(
                        out=w_sb[:, 2 * j * I:(2 * j + 2) * I].rearrange(
                            "p (j c) -> p j c", j=2),
                        in_=w_d[rows, :].rearrange("(j p) c -> p j c", p=P),
                    )

            def load_xt(b):
                tok = slice(b * TB, (b + 1) * TB)
                chunks = []
                for k in range(HK):
                    ch = xtp.tile([P, TB], F32R, tag="xt", name=f"xt{b}_{k}")
                    nc.gpsimd.dma_start(
                        out=ch[:], in_=xt_d[k * P:(k + 1) * P, tok]
                    )
                    chunks.append(ch)
                return chunks

            def router(xt_ch):
                # w[tok] = softmax(logits)[:, e] for one block
                lt = psr.tile([E, TB], F32, tag="rt", name="lt")
                for k in range(HK):
                    nc.tensor.matmul(
                        lt[:],
                        (wr_sb[:, k * E:(k + 1) * E]),
                        (xt_ch[k][:]),
                        start=(k == 0),
                        stop=(k == HK - 1),
                    )
                exp_sb = rtp.tile([E, TB], F32R, tag="exp", name="exp_sb")
                nc.scalar.activation(exp_sb[:], lt[:], AF.Exp)
                w_tiles = []
                for m in range(NM):
                    dn = psr.tile([P, 2], F32, tag="rt", name="dn")
                    nc.tensor.matmul(
                        dn[:],
                        (exp_sb[:, m * P:(m + 1) * P]),
                        (sel_sb[:]),
                        start=True,
                        stop=True,
                    )
                    rec = wp.tile([P, 1], F32, tag="rec", name="rec")
                    nc.vector.reciprocal(rec[:], dn[:, 0:1])
                    w_m = wp.tile([P, 1], F32, tag=f"w{m}", name="w_m")
                    nc.vector.tensor_tensor(
                        out=w_m[:], in0=dn[:, 1:2], in1=rec[:],
                        op=mybir.AluOpType.mult,
                    )
                    w_tiles.append(w_m)
                return w_tiles

            xt_next = load_xt(0)
            w_next = router(xt_next)
            wd_next = None
            for b in range(NB):
                xt_ch = xt_next
                w_tiles = w_next

                # ---- stage 1: hT[i] = silu(G)*U, [I-chunk, tok] layout
                # Block 0 runs a G-pass then a U-pass: the G matmuls only
                # need wg, which finishes streaming ~22us before wu.
                ht_sb = htp.tile([P, IK * TB], F32R, tag="ht")

                def g_step(i):
                    g_ps = psgu.tile([P, TB], F32, tag="gu", name="g_ps")
                    for k in range(HK):
                        nc.tensor.matmul(
                            g_ps[:],
                            (wg_sb[:, k * I + i * P: k * I + (i + 1) * P]),
                            (xt_ch[k][:]),
                            start=(k == 0),
                            stop=(k == HK - 1),
                        )
                    nc.scalar.activation(
                        ht_sb[:, i * TB:(i + 1) * TB], g_ps[:], AF.Silu
                    )

                def u_step(i):
                    u_ps = psgu.tile([P, TB], F32, tag="gu", name="u_ps")
                    for k in range(HK):
                        nc.tensor.matmul(
                            u_ps[:],
                            (wu_sb[:, k * I + i * P: k * I + (i + 1) * P]),
                            (xt_ch[k][:]),
                            start=(k == 0),
                            stop=(k == HK - 1),
                        )
                    hsl = ht_sb[:, i * TB:(i + 1) * TB]
                    nc.vector.tensor_tensor(
                        out=hsl, in0=hsl, in1=u_ps[:], op=mybir.AluOpType.mult
                    )

                if b == 0:
                    for i in range(IK):
                        g_step(i)
                    for i in range(IK):
                        u_step(i)
                else:
                    for i in range(IK):
                        g_step(i)
                        u_step(i)

                if b + 1 < NB:
                    xt_next = load_xt(b + 1)

                # ---- stage 2: Y[m] [128tok, 512h] = hT^T @ Wd, scaled by w
                # wd streamed as 512KB i-pair batches (one DMA fills two
                # 128-row I-chunks side by side in the free dim)
                def load_wd_pair(j, h):
                    wd_t = wdp.tile([P, 1024], F32R, tag="wd", name=f"wd{h}_{j}")
                    src = wd_d[2 * j * P:(2 * j + 2) * P, h * 512:(h + 1) * 512]
                    nc.sync.dma_start(
                        out=wd_t[:].rearrange("p (j c) -> p j c", j=2),
                        in_=src.rearrange("(j p) c -> p j c", p=P),
                    )
                    return wd_t

                y_out = []
                for h in range(NH):
                    if h == 1 and b + 1 < NB:
                        # next block's router between the h-sweeps: fills the
                        # PE while the wd stream for h1 catches up, and takes
                        # the router off the next block-start critical path
                        w_next = router(xt_next)
                    y_ps = [
                        psy.tile([P, 512], F32, tag="y", name=f"y_ps{m}")
                        for m in range(NM)
                    ]
                    for i in range(IK):
                        if i % 2 == 0:
                            j = i // 2
                            if h == 0 and j < len(wd_next or ()):
                                wd_t = wd_next[j]
                            else:
                                wd_t = load_wd_pair(j, h)
                        rhs = wd_t[:, (i % 2) * 512:(i % 2 + 1) * 512]
                        for m in range(NM):
                            nc.tensor.matmul(
                                y_ps[m][:],
                                (ht_sb[:, i * TB + m * P: i * TB + (m + 1) * P]),
                                (rhs),
                                start=(i == 0),
                                stop=(i == IK - 1),
                            )
                    for m in range(NM):
                        y_sb = evp.tile([P, 512], F32, tag="ev", name=f"yev{h}_{m}")
                        nc.vector.tensor_scalar_mul(y_sb[:], y_ps[m][:], w_tiles[m][:])
                        y_out.append((y_sb, m, h))
                wd_next = (
                    [load_wd_pair(0, 0), load_wd_pair(1, 0)]
                    if b + 1 < NB else None
                )
                for y_sb, m, h in y_out:
                    nc.sync.dma_start(
                        out=y_d[b * TB + m * P: b * TB + (m + 1) * P,
                                h * 512:(h + 1) * 512],
                        in_=y_sb[:],
                    )

    nc.compile()
    return nc


def kernel(x, W_router, W_gate, W_up, W_down):
    global LAST_EXEC_NS
    if "nc" not in _CACHE:
        _CACHE["nc"] = _build()
    nc = _CACHE["nc"]

    xt = np.ascontiguousarray(x.reshape(N, H).T).astype(np.float32, copy=False)
    wr = np.ascontiguousarray(W_router).astype(np.float32, copy=False)
    eye = np.eye(E, dtype=np.float32)
    in_maps = []
    for e in range(E):
        sel = np.stack([np.ones(E, dtype=np.float32), eye[e]], axis=1)
        in_maps.append({
            "xt": xt,
            "wg": np.ascontiguousarray(W_gate[e]).astype(np.float32, copy=False),
            "wu": np.ascontiguousarray(W_up[e]).astype(np.float32, copy=False),
            "wd": np.ascontiguousarray(W_down[e]).astype(np.float32, copy=False),
            "wr": wr,
            "sel": np.ascontiguousarray(sel),
        })

    res = run_bass_kernel_spmd(nc, in_maps, list(range(E)), trace=TRACE)
    LAST_EXEC_NS = res.exec_time_ns

    acc = np.zeros((N, H), dtype=np.float64)
    for r in res.results:
        acc += r["y"]
    return acc.astype(np.float32).reshape(x.shape[0], x.shape[1], H)



# revision 5
# speedup vs baseline: 1.1380x; 1.1380x over previous
"""MoE layer (dense all-experts SwiGLU + router-weighted sum) on 8 TRN2 cores.

Expert-parallel: core e holds expert e's weights (E=8). Every core sees the
full token stream x (shipped pre-transposed as xt [H, N]) and computes
  y_e = softmax(x @ W_router)[:, e] * ((silu(x@Wg_e) * (x@Wu_e)) @ Wd_e)
The host sums the 8 per-expert outputs.

v2 vs the fp32r baseline (1583us):
  - all matmul operands in bf16 (same 1 col/cycle PE rate as f32r, ~2e-3
    extra rel err, far under the 2e-2 gate). Halves SBUF+DMA so ALL
    weights (wg/wu/wd = 12MB) are SBUF-resident: the per-block 8MB wd
    re-stream is gone and stage 2 never waits on DMA.
  - bf16 stationary weights get FWL (2x faster LDWEIGHTS), so the
    weight-load pipeline gap per matmul shrinks.
  - host pre-arranges weights into the exact SBUF layouts so every weight
    DMA is a contiguous copy; wg/wu are streamed in i-stripe order so
    g_step(0) can start after the first 256KB stripe lands (~1us), not
    after the full 8MB (~22us).
  - silu via the exp-family table: silu(g)*u = (g*u)*(0.5*tanh(0.5 g)+0.5).
    The baseline alternated Silu/Exp activation tables every block: 32
    ACT_TABLE_LOADs (~1.3us each) that stalled PSUM eviction at each
    block start. Tanh and Exp share one table set -> 1 load total.
  - ht double-buffered across blocks (kills the write-after-read hazard
    that serialized block b+1's stage-1 eviction behind block b's stage 2).
  - router split: logits+exp emitted between the two stage-2 h-sweeps,
    the tiny denominator/weight matmuls emitted after h1's first i-chunk
    so the PE never idles waiting on ScalarE's exp.

Per-core program, per 512-token block (PE work = 2*16*8 + 2*4*16 = 384
F=512 matmuls = 81.9us at 2.4GHz; DMA 0.5MB in + 2MB out; DVE ~14us;
ScalarE ~23us - PE-bound with wide margins everywhere else).
"""
import numpy as np
import ml_dtypes

import concourse.bass as bass
import concourse.mybir as mybir
import concourse.tile as tile
from concourse import bacc
from concourse.bass_utils import run_bass_kernel_spmd

P = 128
H, I, E = 1024, 2048, 8
N = 8192       # tokens = 4 * 2048
HK = H // P    # 8 contraction chunks over H
IK = I // P    # 16 chunks over I
TB = 512       # token block
NB = N // TB   # 16 blocks
NM = TB // P   # 4 token subtiles per block
NH = H // 512  # 2 output column halves
SW = HK * P    # wg/wu stripe width: stripe i holds cols i*SW..(i+1)*SW

F32 = mybir.dt.float32
F32R = mybir.dt.float32r
BF16 = mybir.dt.bfloat16
AF = mybir.ActivationFunctionType
MUL = mybir.AluOpType.mult
ADD = mybir.AluOpType.add

# set by a driver (test.py) to profile; harness path keeps defaults
TRACE = False
LAST_EXEC_NS = None

_CACHE = {}


def _build():
    nc = bacc.Bacc("TRN2", target_bir_lowering=False, debug=False)

    xt_d = nc.dram_tensor("xt", [H, N], BF16, kind="ExternalInput").ap()
    wg_d = nc.dram_tensor("wg", [P, IK * SW], BF16, kind="ExternalInput").ap()
    wu_d = nc.dram_tensor("wu", [P, IK * SW], BF16, kind="ExternalInput").ap()
    wd_d = nc.dram_tensor("wd", [P, IK * H], BF16, kind="ExternalInput").ap()
    wr_d = nc.dram_tensor("wr", [P, HK * E], BF16, kind="ExternalInput").ap()
    sel_d = nc.dram_tensor("sel", [E, 2], F32R, kind="ExternalInput").ap()
    y_d = nc.dram_tensor("y", [N, H], F32, kind="ExternalOutput").ap()

    with tile.TileContext(nc) as tc:
        with (
            tc.tile_pool(name="const", bufs=1) as const,
            tc.tile_pool(name="xtp", bufs=24) as xtp,
            tc.tile_pool(name="htp", bufs=2) as htp,
            tc.tile_pool(name="s1p", bufs=6) as s1p,
            tc.tile_pool(name="evp", bufs=4) as evp,
            tc.tile_pool(name="rtp", bufs=2) as rtp,
            tc.tile_pool(name="wp", bufs=2) as wp,
            tc.tile_pool(name="psgu", bufs=3, space="PSUM") as psgu,
            tc.tile_pool(name="psy", bufs=4, space="PSUM") as psy,
            tc.tile_pool(name="psr", bufs=1, space="PSUM") as psr,
        ):
            # resident weights, in the exact host-prearranged layouts
            wg_sb = const.tile([P, IK * SW], BF16)
            wu_sb = const.tile([P, IK * SW], BF16)
            wd_sb = const.tile([P, IK * H], BF16)
            wr_sb = const.tile([P, HK * E], BF16)
            sel_sb = const.tile([E, 2], F32R)
            nc.sync.dma_start(out=wr_sb[:], in_=wr_d[:])
            nc.sync.dma_start(out=sel_sb[:], in_=sel_d[:])
            # wg/wu interleaved per 256KB i-stripe: g_step(i)/u_step(i) only
            # need stripe i, so the PE starts ~1us in instead of ~22us
            for i in range(IK):
                cols = slice(i * SW, (i + 1) * SW)
                nc.sync.dma_start(out=wg_sb[:, cols], in_=wg_d[:, cols])
                nc.sync.dma_start(out=wu_sb[:, cols], in_=wu_d[:, cols])
            # wd needed only from block-0 stage 2 (~55us in); 4x 1MB
            for q in range(4):
                cols = slice(q * 4 * H, (q + 1) * 4 * H)
                nc.sync.dma_start(out=wd_sb[:, cols], in_=wd_d[:, cols])

            def load_xt(b):
                tok = slice(b * TB, (b + 1) * TB)
                chunks = []
                for k in range(HK):
                    ch = xtp.tile([P, TB], BF16, tag="xt", name=f"xt{b}_{k}")
                    nc.gpsimd.dma_start(
                        out=ch[:], in_=xt_d[k * P:(k + 1) * P, tok]
                    )
                    chunks.append(ch)
                return chunks

            def router_logits(xt_ch):
                # exp(logits)^T [8, TB] for one block
                lt = psr.tile([E, TB], F32, tag="rt", name="lt")
                for k in range(HK):
                    nc.tensor.matmul(
                        lt[:],
                        (wr_sb[:, k * E:(k + 1) * E]),
                        (xt_ch[k][:]),
                        start=(k == 0),
                        stop=(k == HK - 1),
                    )
                exp_sb = rtp.tile([E, TB], F32R, tag="exp", name="exp_sb")
                nc.scalar.activation(exp_sb[:], lt[:], AF.Exp)
                return exp_sb

            def router_weights(exp_sb):
                # w[tok] = exp_e / sum_e' exp_e' via per-subtile transpose-mm.
                # All 4 [denom|numer] matmuls land in one PSUM tile so they
                # issue back-to-back on the PE with no DVE dependency between
                # them (psr has one bank; separate tiles would serialize
                # matmul m+1 behind reciprocal m).
                dn = psr.tile([P, 2 * NM], F32, tag="rt", name="dn")
                for m in range(NM):
                    nc.tensor.matmul(
                        dn[:, 2 * m:2 * m + 2],
                        (exp_sb[:, m * P:(m + 1) * P]),
                        (sel_sb[:]),
                        start=True,
                        stop=True,
                    )
                w_tiles = []
                for m in range(NM):
                    rec = wp.tile([P, 1], F32, tag="rec", name="rec")
                    nc.vector.reciprocal(rec[:], dn[:, 2 * m:2 * m + 1])
                    w_m = wp.tile([P, 1], F32, tag=f"w{m}", name="w_m")
                    nc.vector.tensor_tensor(
                        out=w_m[:], in0=dn[:, 2 * m + 1:2 * m + 2], in1=rec[:],
                        op=MUL,
                    )
                    w_tiles.append(w_m)
                return w_tiles

            xt_next = load_xt(0)
            xt_next2 = load_xt(1)
            exp0 = router_logits(xt_next)
            w_next = router_weights(exp0)

            for b in range(NB):
                xt_ch = xt_next
                w_tiles = w_next

                # ---- stage 1: hT[i] = silu(G)*U = (G*U)*(tanh(G/2)/2+1/2),
                # [I-chunk, tok] layout. Tanh shares the Exp table set ->
                # no ACT_TABLE_LOAD thrash (the baseline's Silu forced 2
                # table swaps per block).
                # ht holds 2*silu(g)*u = p + p*tanh(g/2), p = g*u (the 1/2 is
                # folded into wd host-side). Both ScalarE evictions of g_ps
                # (tanh + copy) run during the U matmuls, so g's PSUM bank
                # frees one matmul-group early - the psgu rotation then never
                # backpressures the PE (this was ~8x 432ns PE hiccups/block).
                ht_sb = htp.tile([P, IK * TB], BF16, tag="ht")
                for i in range(IK):
                    g_ps = psgu.tile([P, TB], F32, tag="gu", name="g_ps")
                    for k in range(HK):
                        nc.tensor.matmul(
                            g_ps[:],
                            (wg_sb[:, i * SW + k * P: i * SW + (k + 1) * P]),
                            (xt_ch[k][:]),
                            start=(k == 0),
                            stop=(k == HK - 1),
                        )
                    th = s1p.tile([P, TB], BF16, tag="s1", name="th")
                    nc.scalar.activation(th[:], g_ps[:], AF.Tanh, scale=0.5)
                    g_sb = s1p.tile([P, TB], BF16, tag="s1", name="g_sb")
                    nc.scalar.activation(g_sb[:], g_ps[:], AF.Copy)
                    u_ps = psgu.tile([P, TB], F32, tag="gu", name="u_ps")
                    for k in range(HK):
                        nc.tensor.matmul(
                            u_ps[:],
                            (wu_sb[:, i * SW + k * P: i * SW + (k + 1) * P]),
                            (xt_ch[k][:]),
                            start=(k == 0),
                            stop=(k == HK - 1),
                        )
                    p_sb = s1p.tile([P, TB], BF16, tag="s1", name="p_sb")
                    nc.vector.tensor_tensor(
                        out=p_sb[:], in0=u_ps[:], in1=g_sb[:], op=MUL,
                    )
                    t_sb = s1p.tile([P, TB], BF16, tag="s1", name="t_sb")
                    nc.vector.tensor_tensor(
                        out=t_sb[:], in0=p_sb[:], in1=th[:], op=MUL,
                    )
                    nc.vector.tensor_tensor(
                        out=ht_sb[:, i * TB:(i + 1) * TB],
                        in0=t_sb[:], in1=p_sb[:], op=ADD,
                    )

                if b + 2 < NB:
                    xt_next2_new = load_xt(b + 2)
                xt_next = xt_next2
                if b + 2 < NB:
                    xt_next2 = xt_next2_new

                # ---- stage 2: Y[m] [128tok, 512h] = hT^T @ Wd, scaled by w.
                # Router for block b+1 rides between/inside the h-sweeps.
                y_out = []
                exp_next = None
                for h in range(NH):
                    if h == 1 and b + 1 < NB:
                        exp_next = router_logits(xt_next)
                    y_ps = [
                        psy.tile([P, 512], F32, tag="y", name=f"y_ps{m}")
                        for m in range(NM)
                    ]
                    for i in range(IK):
                        rhs = wd_sb[:, i * H + h * 512: i * H + (h + 1) * 512]
                        for m in range(NM):
                            nc.tensor.matmul(
                                y_ps[m][:],
                                (ht_sb[:, i * TB + m * P: i * TB + (m + 1) * P]),
                                (rhs),
                                start=(i == 0),
                                stop=(i == IK - 1),
                            )
                        if h == 1 and i == 0 and exp_next is not None:
                            # tiny denominator matmuls after h1's first
                            # i-chunk: exp (ScalarE) has had 4 matmul-times
                            # to finish, so the PE never waits on it
                            w_next = router_weights(exp_next)
                    for m in range(NM):
                        y_sb = evp.tile([P, 512], F32, tag="ev", name=f"yev{h}_{m}")
                        nc.vector.tensor_scalar_mul(y_sb[:], y_ps[m][:], w_tiles[m][:])
                        y_out.append((y_sb, m, h))
                        nc.sync.dma_start(
                            out=y_d[b * TB + m * P: b * TB + (m + 1) * P,
                                    h * 512:(h + 1) * 512],
                            in_=y_sb[:],
                        )

    nc.compile()
    return nc


def kernel(x, W_router, W_gate, W_up, W_down):
    global LAST_EXEC_NS
    if "nc" not in _CACHE:
        _CACHE["nc"] = _build()
    nc = _CACHE["nc"]

    bf16 = ml_dtypes.bfloat16
    xt = np.ascontiguousarray(
        np.asarray(x, dtype=np.float32).reshape(N, H).T
    ).astype(bf16)
    wr = np.ascontiguousarray(
        np.asarray(W_router, dtype=np.float32)
        .reshape(HK, P, E).transpose(1, 0, 2).reshape(P, HK * E)
    ).astype(bf16)
    eye = np.eye(E, dtype=np.float32)
    in_maps = []
    for e in range(E):
        sel = np.stack([np.ones(E, dtype=np.float32), eye[e]], axis=1)
        wg = (
            np.asarray(W_gate[e], dtype=np.float32)
            .reshape(HK, P, IK, P).transpose(1, 2, 0, 3).reshape(P, IK * SW)
        )
        wu = (
            np.asarray(W_up[e], dtype=np.float32)
            .reshape(HK, P, IK, P).transpose(1, 2, 0, 3).reshape(P, IK * SW)
        )
        # 0.5x folds the (1+tanh)/2 normalization of stage 1 into wd
        wd = (
            np.asarray(W_down[e], dtype=np.float32)
            .reshape(IK, P, H).transpose(1, 0, 2).reshape(P, IK * H)
        ) * 0.5
        in_maps.append({
            "xt": xt,
            "wg": np.ascontiguousarray(wg).astype(bf16),
            "wu": np.ascontiguousarray(wu).astype(bf16),
            "wd": np.ascontiguousarray(wd).astype(bf16),
            "wr": wr,
            "sel": np.ascontiguousarray(sel),
        })

    res = run_bass_kernel_spmd(nc, in_maps, list(range(E)), trace=TRACE)
    LAST_EXEC_NS = res.exec_time_ns

    acc = np.zeros((N, H), dtype=np.float64)
    for r in res.results:
        acc += r["y"]
    return acc.astype(np.float32).reshape(x.shape[0], x.shape[1], H)
